# revision 1
# baseline (speedup 1.0000x reference)
"""LID detector kernel for Trainium2 (8 NeuronCores, data-parallel over batch).

Per core (batch shard of 32):
  - mean-pool each feature map over space -> q [C, 32] (transposed layout)
  - -d2 = 2*q.r - ||r||^2 - ||q||^2 via PE matmuls into PSUM, evicted into a
    stacked [128, 2000] buffer (partition quadrant = layer)
  - top-24 smallest d2 via 3 rounds of DVE max8 + match_replace
  - LID = -2k / (sum_{i=1..20} ln d2_i - 20 ln d2_20)  (no sqrt needed)
  - logit = w . lid + b -> sigmoid -> out [32]
"""

import sys

for _p in ("/opt/trn_rl_repo", "/root/.axon_site/_ro/trn_rl_repo"):
    if _p not in sys.path:
        sys.path.append(_p)

import ml_dtypes
import numpy as np

import concourse.mybir as mybir
from concourse import bass, bacc
from concourse.tile import TileContext
from concourse.bass_utils import run_bass_kernel_spmd

F32 = mybir.dt.float32
BF16 = mybir.dt.bfloat16
N_CORES = 8
B = 32  # batch shard per core
R = 2000
K = 20
LAYERS = [(64, 3136), (128, 784), (256, 196), (512, 49)]  # (C, H*W)
NEG_BIG = -3.0e38

# column j of qT holds sample SIGMA[j] of the local shard
SIGMA = np.array([2 * j for j in range(16)] + [2 * j + 1 for j in range(16)])


def build_nc():
    nc = bacc.Bacc("TRN2", target_bir_lowering=False, debug=False,
                   num_devices=N_CORES)

    feats = [nc.dram_tensor(f"feat{l}", [B, C, HW], BF16, kind="ExternalInput")
             for l, (C, HW) in enumerate(LAYERS)]
    refTs = [nc.dram_tensor(f"refT{l}", [C, R], F32, kind="ExternalInput")
             for l, (C, _) in enumerate(LAYERS)]
    regw = nc.dram_tensor("regw", [1, 4], F32, kind="ExternalInput")
    regb = nc.dram_tensor("regb", [1, 1], F32, kind="ExternalInput")
    out = nc.dram_tensor("out", [B, 1], F32, kind="ExternalOutput")
    import os
    _dbg = os.environ.get("DEBUG_LID") == "1"
    if _dbg:
        dbg_lid = nc.dram_tensor("dbg_lid", [128, 1], F32, kind="ExternalOutput")
        dbg_vals = nc.dram_tensor("dbg_vals", [128, 24], F32, kind="ExternalOutput")
        dbg_q = nc.dram_tensor("dbg_q", [64, B], F32, kind="ExternalOutput")
        dbg_tk = nc.dram_tensor("dbg_tk", [128, R], F32, kind="ExternalOutput")
        dbg_rn2a = nc.dram_tensor("dbg_rn2a", [65, R], F32, kind="ExternalOutput")
        dbg_rn2b = nc.dram_tensor("dbg_rn2b", [1, R], F32, kind="ExternalOutput")

    with TileContext(nc) as tc:
        with (
            tc.tile_pool(name="persist", bufs=1) as pp,
            tc.tile_pool(name="ft", bufs=6) as fp,
            tc.tile_pool(name="sq", bufs=2) as sqp,
        ):
            # ---- persistent tiles
            rt = {}   # (l, i) -> refT chunk tile [Cc, R]
            for l, (C, _) in enumerate(LAYERS):
                for i in range(0, C, 128):
                    Cc = min(128, C - i)
                    rt[(l, i)] = pp.tile([Cc, R], F32, tag=f"rt{l}_{i}",
                                         name=f"rt{l}_{i}")
            act_scratch = pp.tile([128, 3136], BF16, tag="act_scratch",
                                  name="act_scratch")
            rn2a = pp.tile([65, R], F32, tag="rn2a", name="rn2a")
            rn2b = pp.tile([1, R], F32, tag="rn2b", name="rn2b")
            rn2base = [(rn2a, 0), (rn2a, 32), (rn2a, 64), (rn2b, 0)]
            rn2row = [t[b:b + 1, :] for (t, b) in rn2base]
            qT = {}
            for l, (C, _) in enumerate(LAYERS):
                for i in range(0, C, 128):
                    Cc = min(128, C - i)
                    qT[(l, i)] = pp.tile([Cc, B], F32, tag=f"qT{l}_{i}", name=f"qT{l}_{i}")
            qn2neg = [pp.tile([B, 1], F32, tag=f"qn2_{l}", name=f"qn2_{l}") for l in range(4)]
            topkbuf = pp.tile([128, R], F32, tag="topkbuf", name="topkbuf")
            vals = pp.tile([128, 24], F32, tag="vals", name="vals")
            ones_col = pp.tile([128, 1], F32, tag="ones_col", name="ones_col")
            ones_row = pp.tile([1, B], F32, tag="ones_row", name="ones_row")
            negones_all = pp.tile([65, B], F32, tag="negones_all", name="negones_all")
            wb_sb = pp.tile([1, 5], F32, tag="wb_sb", name="wb_sb")
            tmp0 = pp.tile([128, 16], F32, tag="tmp0", name="tmp0")

            nc.vector.memset(ones_col[:], 1.0)
            nc.vector.memset(ones_row[:], 1.0)
            nc.vector.memset(negones_all[:], -1.0)
            nc.sync.dma_start(out=wb_sb[0:1, 0:4], in_=regw[:])
            nc.sync.dma_start(out=wb_sb[0:1, 4:5], in_=regb[:])

            # ---- ref squared norms: rn2[l] [1, R] = sum_c refT^2
            with tc.tile_pool(name="psumA", bufs=1,
                              space=bass.MemorySpace.PSUM) as pA:
                for l, (C, _) in enumerate(LAYERS):
                    ps = pA.tile([1, R], F32, tag="rn2ps", name="rn2ps")
                    chunks = list(range(0, C, 128))
                    for ci, i in enumerate(chunks):
                        Cc = min(128, C - i)
                        sq = sqp.tile([128, R], F32, tag="sq", name="sq")
                        nc.scalar.square(sq[0:Cc, :], rt[(l, i)][:])
                        for c0 in range(0, R, 512):
                            n = min(512, R - c0)
                            nc.tensor.matmul(
                                ps[0:1, c0:c0 + n],
                                ones_col[0:Cc, 0:1],
                                sq[0:Cc, c0:c0 + n],
                                start=(ci == 0), stop=(ci == len(chunks) - 1),
                            )
                    nc.scalar.copy(rn2row[l], ps[:])

            # ---- pooling: fill qT columns (sample order SIGMA)
            # layer 0: C=64, 2 samples per 128 partitions
            C, HW = LAYERS[0]
            for t in range(8):
                tile = fp.tile([128, 2, HW], BF16, tag="ft", name="ft")
                src = bass.AP(feats[0], 4 * t * C * HW,
                              [[HW, 128], [2 * C * HW, 2], [1, HW]])
                nc.sync.dma_start(out=tile[:], in_=src)
                if t < 4:
                    nc.vector.tensor_reduce(
                        tmp0[:, 2 * t:2 * t + 2], tile[:],
                        axis=mybir.AxisListType.X, op=mybir.AluOpType.add)
                else:
                    for g in range(2):
                        nc.scalar.activation(
                            act_scratch[:, 0:HW], tile[:, g, :],
                            mybir.ActivationFunctionType.Copy,
                            accum_out=tmp0[:, 2 * t + g:2 * t + g + 1])
            nc.vector.tensor_copy(qT[(0, 0)][:, 0:16], tmp0[0:64, :])
            nc.vector.tensor_copy(qT[(0, 0)][:, 16:32], tmp0[64:128, :])

            # layers 1..3: per chunk, DMA samples with stride 2 (even then odd)
            for l in (1, 2, 3):
                C, HW = LAYERS[l]
                n_chunks = C // 128
                g = B // (2 * 4 // 1)  # placeholder, set below
                # samples per DMA: L1:4 (8 DMAs), L2:8 (4 DMAs), L3:16 (2 DMAs)
                spd = {1: 4, 2: 8, 3: 16}[l]
                ndma = B // spd
                for i in range(n_chunks):
                    for t in range(ndma):
                        # cols spd*t .. spd*t+spd-1 -> samples SIGMA[col]
                        # = base + 2*j, base = 2*spd*t if even half else ...
                        col0 = spd * t
                        s_base = int(SIGMA[col0])
                        tile = fp.tile([128, spd, HW], BF16, tag="ft", name="ft")
                        src = bass.AP(
                            feats[l],
                            s_base * C * HW + 128 * i * HW,
                            [[HW, 128], [2 * C * HW, spd], [1, HW]])
                        nc.sync.dma_start(out=tile[:], in_=src)
                        if l == 1:
                            for g in range(spd):
                                nc.scalar.activation(
                                    act_scratch[:, 0:HW], tile[:, g, :],
                                    mybir.ActivationFunctionType.Copy,
                                    accum_out=qT[(l, 128 * i)][:, col0 + g:col0 + g + 1])
                        else:
                            nc.vector.tensor_reduce(
                                qT[(l, 128 * i)][:, col0:col0 + spd], tile[:],
                                axis=mybir.AxisListType.X, op=mybir.AluOpType.add)

            for l, (C, _) in enumerate(LAYERS):
                for i in range(0, C, 128):
                    Cc = min(128, C - i)
                    nc.sync.dma_start(out=rt[(l, i)][:],
                                      in_=refTs[l][i:i + Cc, :])

            # ---- scale qT by 2/HW (so lhsT holds 2*q), qn2neg
            with tc.tile_pool(name="psumB", bufs=1,
                              space=bass.MemorySpace.PSUM) as pB:
                for l, (C, HW) in enumerate(LAYERS):
                    chunks = list(range(0, C, 128))
                    qps = pB.tile([B, 1], F32, tag="qn2ps", name="qn2ps")
                    for ci, i in enumerate(chunks):
                        Cc = min(128, C - i)
                        nc.scalar.mul(qT[(l, i)][:], qT[(l, i)][:], 2.0 / HW)
                        qsq = sqp.tile([128, B], F32, tag="qsq", name="qsq")
                        # (2q * 0.5)^2 = q^2
                        nc.scalar.activation(
                            qsq[0:Cc, :], qT[(l, i)][:],
                            mybir.ActivationFunctionType.Square, scale=0.5)
                        nc.tensor.matmul(
                            qps[:], qsq[0:Cc, :], ones_col[0:Cc, 0:1],
                            start=(ci == 0), stop=(ci == len(chunks) - 1))
                    nc.scalar.mul(qn2neg[l][:], qps[:], -1.0)

                # ---- distances: psum = 2q.r - rn2 ; evict + qn2neg -> -d2
                for l, (C, _) in enumerate(LAYERS):
                    chunks = list(range(0, C, 128))
                    for c0 in range(0, R, 512):
                        n = min(512, R - c0)
                        dps = pB.tile([B, 512], F32, tag="d2ps", name="d2ps")
                        for ci, i in enumerate(chunks):
                            Cc = min(128, C - i)
                            nc.tensor.matmul(
                                dps[:, 0:n], qT[(l, i)][:],
                                rt[(l, i)][:, c0:c0 + n],
                                start=(ci == 0), stop=False)
                        rn2t, rn2b_ = rn2base[l]
                        nc.tensor.matmul(
                            dps[:, 0:n], negones_all[rn2b_:rn2b_ + 1, :],
                            rn2t[rn2b_:rn2b_ + 1, c0:c0 + n],
                            start=False, stop=True)
                        nc.vector.tensor_scalar(
                            topkbuf[32 * l:32 * l + 32, c0:c0 + n],
                            dps[:, 0:n], qn2neg[l][:], None,
                            op0=mybir.AluOpType.add)

                if _dbg:
                    nc.sync.dma_start(out=dbg_tk[:], in_=topkbuf[:])
                    nc.sync.dma_start(out=dbg_rn2a[:], in_=rn2a[:])
                    nc.sync.dma_start(out=dbg_rn2b[:], in_=rn2b[:])
                # ---- top-24 (ascending d2 == descending -d2)
                nc.vector.max(vals[:, 0:8], topkbuf[:])
                nc.vector.match_replace(topkbuf[:], vals[:, 0:8], topkbuf[:],
                                        NEG_BIG)
                nc.vector.max(vals[:, 8:16], topkbuf[:])
                nc.vector.match_replace(topkbuf[:], vals[:, 8:16], topkbuf[:],
                                        NEG_BIG)
                nc.vector.max(vals[:, 16:24], topkbuf[:])

                # ---- LID
                ln2 = pp.tile([128, 24], F32, tag="ln2", name="ln2")
                S = pp.tile([128, 1], F32, tag="S", name="S")
                denom = pp.tile([128, 1], F32, tag="denom", name="denom")
                lid = pp.tile([128, 1], F32, tag="lid", name="lid")
                # clamp: vals <= -1e-30 so that -vals >= 1e-30
                nc.vector.tensor_scalar_min(vals[:], vals[:], -1e-30)
                nc.scalar.activation(ln2[:], vals[:],
                                     mybir.ActivationFunctionType.Ln,
                                     scale=-1.0)
                nc.vector.tensor_reduce(S[:], ln2[:, 1:21],
                                        axis=mybir.AxisListType.X,
                                        op=mybir.AluOpType.add)
                # denom = -20*ln2[20] + S  (= sum ln d2_i - 20 ln d2_20)
                nc.vector.tensor_scalar(denom[:], ln2[:, 20:21], -20.0, S[:],
                                        op0=mybir.AluOpType.mult,
                                        op1=mybir.AluOpType.add)
                nc.vector.reciprocal(lid[:], denom[:])
                nc.vector.tensor_scalar_mul(lid[:], lid[:], -2.0 * K)
                if _dbg:
                    nc.sync.dma_start(out=dbg_lid[:], in_=lid[:])
                    nc.sync.dma_start(out=dbg_vals[:], in_=vals[:])
                    nc.sync.dma_start(out=dbg_q[:], in_=qT[(0, 0)][:])

                # ---- regression + sigmoid
                lid4 = pp.tile([B, 4], F32, tag="lid4", name="lid4")
                for l in range(4):
                    nc.vector.tensor_copy(lid4[:, l:l + 1],
                                          lid[32 * l:32 * l + 32, :])
                wps = pB.tile([B, 5], F32, tag="wps", name="wps")
                nc.tensor.matmul(wps[:], ones_row[:], wb_sb[:],
                                 start=True, stop=True)
                wbc = pp.tile([B, 5], F32, tag="wbc", name="wbc")
                nc.scalar.copy(wbc[:], wps[:])
                prod = pp.tile([B, 4], F32, tag="prod", name="prod")
                nc.vector.tensor_tensor(prod[:], lid4[:], wbc[:, 0:4],
                                        op=mybir.AluOpType.mult)
                ssum = pp.tile([B, 1], F32, tag="ssum", name="ssum")
                nc.vector.tensor_reduce(ssum[:], prod[:],
                                        axis=mybir.AxisListType.X,
                                        op=mybir.AluOpType.add)
                res = pp.tile([B, 1], F32, tag="res", name="res")
                nc.scalar.activation(res[:], ssum[:],
                                     mybir.ActivationFunctionType.Sigmoid,
                                     bias=wbc[:, 4:5])
                nc.sync.dma_start(out=out[:], in_=res[:])

    nc.compile()
    return nc


_NC = None


def _get_nc():
    global _NC
    if _NC is None:
        _NC = build_nc()
    return _NC


def run(trace=False, **inputs):
    nc = _get_nc()
    feats = [np.asarray(inputs[f"feat{l}"], dtype=np.float32) for l in range(4)]
    refTs = [np.ascontiguousarray(np.asarray(inputs[f"ref{l}"],
                                             dtype=np.float32).T)
             for l in range(4)]
    regw = np.asarray(inputs["reg_w"], dtype=np.float32).reshape(1, 4)
    regb = np.asarray(inputs["reg_b"], dtype=np.float32).reshape(1, 1)
    assert int(inputs.get("k", K)) == K

    in_maps = []
    for c in range(N_CORES):
        m = {}
        for l, (C, HW) in enumerate(LAYERS):
            m[f"feat{l}"] = np.ascontiguousarray(
                feats[l][c * B:(c + 1) * B].reshape(B, C, HW)).astype(
                    ml_dtypes.bfloat16)
            m[f"refT{l}"] = refTs[l]
        m["regw"] = regw
        m["regb"] = regb
        in_maps.append(m)

    res = run_bass_kernel_spmd(nc, in_maps, core_ids=list(range(N_CORES)),
                               trace=trace)
    full = np.empty((N_CORES * B,), dtype=np.float32)
    for c in range(N_CORES):
        shard = np.empty((B,), dtype=np.float32)
        shard[SIGMA] = res.results[c]["out"][:, 0]
        full[c * B:(c + 1) * B] = shard
    return full, res


def kernel(**inputs):
    return run(trace=False, **inputs)[0]



# revision 22
# speedup vs baseline: 2.1351x; 2.1351x over previous
"""LID detector kernel for Trainium2 (8 NeuronCores, data-parallel over batch).

Per core (batch shard of 32 samples):
  - features arrive host-transposed [B, HW, C] in fp8-e4m3; spatial mean
    pooling is done on the TensorEngine as ones-vector matmuls (reduction
    over the partition axis = hw), accumulating q directly in [C, B] layout
    in PSUM.
  - reference tables arrive host-transposed [C, R] fp8; rn2 = ||r||^2 via
    square (ACT/gpsimd) + ones matmul.
  - -d2 = 2q.r - rn2 - qn2 accumulated fully in PSUM: per (layer, chunk)
    C-chunk matmuls plus one K=2 matmul with lhsT [[-1...],[qn2neg]] and
    rhs [[rn2],[ones]]; eviction is then a plain copy into the topk buffer.
  - top-24 smallest d2 via DVE max8 + match_replace on two column halves,
    then a 48-wide merge; LID = -2k / (sum_{i=2..21} ln d2_i - 20 ln d2_21).
  - logit = w . lid + b -> sigmoid -> out [32].
Sample order inside a core is PERM (evens then odds) to allow 2-sample
packing of the layer-0 pooling matmuls; the host inverts it on gather.
"""

import sys

for _p in ("/opt/trn_rl_repo", "/root/.axon_site/_ro/trn_rl_repo"):
    if _p not in sys.path:
        sys.path.append(_p)

import ml_dtypes
import numpy as np

import concourse.mybir as mybir
from concourse import bass, bacc
from concourse.tile import TileContext
from concourse.bass_utils import run_bass_kernel_spmd

F32 = mybir.dt.float32
BF16 = mybir.dt.bfloat16
FP8 = mybir.dt.float8e4
NP_FP8 = ml_dtypes.float8_e4m3
NP_BF16 = ml_dtypes.bfloat16

N_CORES = 8
B = 32          # batch shard per core
R = 2000
K = 20
NEG_BIG = -3.0e38
LAYERS = [(64, 3136), (128, 784), (256, 196), (512, 49)]  # (C, H*W)

# column j of the on-device layout holds sample PERM[j] of the local shard
PERM = np.array([2 * j for j in range(16)] + [2 * j + 1 for j in range(16)])


def build_nc():
    nc = bacc.Bacc("TRN2", target_bir_lowering=False, debug=False,
                   num_devices=N_CORES)

    # feat0 is pair-interleaved: [16, HW, 2*C] with (sample-pair, hw, 2, C)
    feats = [nc.dram_tensor(
        "feat0" if l == 0 else f"feat{l}",
        [B // 2, HW, 2 * C] if l == 0 else [B, HW, C],
        FP8, kind="ExternalInput") for l, (C, HW) in enumerate(LAYERS)]
    rts = [nc.dram_tensor(f"rt{l}", [C, R], FP8, kind="ExternalInput")
           for l, (C, _) in enumerate(LAYERS)]
    regw = nc.dram_tensor("regw", [1, 4], F32, kind="ExternalInput")
    regb = nc.dram_tensor("regb", [1, 1], F32, kind="ExternalInput")
    out = nc.dram_tensor("out", [B, 1], F32, kind="ExternalOutput")

    # rt chunk list per layer: (layer, chunk_index, C0)
    rt_chunks = {l: [(l, i, c0) for i, c0 in enumerate(range(0, C, 128))]
                 for l, (C, _) in enumerate(LAYERS)}
    # distance column chunks
    COL = [(0, 512), (512, 512), (1024, 512), (1536, 464)]

    with TileContext(nc) as tc:
        with (
            tc.tile_pool(name="pp", bufs=1) as pp,
            tc.tile_pool(name="sq", bufs=4) as sqp,
            tc.tile_pool(name="pR", bufs=2, space=bass.MemorySpace.PSUM) as pR,
            tc.tile_pool(name="pQ", bufs=1, space=bass.MemorySpace.PSUM) as pQ,
            tc.tile_pool(name="pD", bufs=2, space=bass.MemorySpace.PSUM) as pD,
        ):
            # ---------------- persistent SBUF tiles
            ones8 = pp.tile([128, 1], FP8, tag="ones8", name="ones8")
            onesb = pp.tile([128, 1], BF16, tag="onesb", name="onesb")
            ones_row = pp.tile([1, B], F32, tag="ones_row", name="ones_row")
            nc.vector.memset(ones8[:], 1.0)
            nc.vector.memset(onesb[:], 1.0)
            nc.vector.memset(ones_row[:], 1.0)

            wb_sb = pp.tile([1, 5], F32, tag="wb_sb", name="wb_sb")
            nc.sync.dma_start(out=wb_sb[0:1, 0:4], in_=regw[:])
            nc.sync.dma_start(out=wb_sb[0:1, 4:5], in_=regb[:])

            # ref tables: one [<=128, R] fp8 tile per C-chunk
            rt = {}
            for l, (C, _) in enumerate(LAYERS):
                for _, i, c0 in rt_chunks[l]:
                    Cc = min(128, C - c0)
                    rt[(l, i)] = pp.tile([Cc, R], FP8, tag=f"rt{l}_{i}",
                                         name=f"rt{l}_{i}")
            # combo-matmul operands (engines may only write partition 0; the
            # partition-1 rows are filled by small SBUF->SBUF DMAs):
            #   lhsT qc*: row0 = -qn2 per sample, row1 = -1
            #   rhs  rc_all: row0 = ones, row1 = rn2  (cols l*R + c)
            rn2sb = pp.tile([1, 4 * R], BF16, tag="rn2sb", name="rn2sb")
            rc_all = pp.tile([2, 4 * R], BF16, tag="rc_all", name="rc_all")
            qc123 = pp.tile([2, 96], BF16, tag="qc123", name="qc123")
            qc0 = pp.tile([2, B], BF16, tag="qc0", name="qc0")
            negones = pp.tile([1, 96], BF16, tag="negones", name="negones")
            nc.vector.memset(rc_all[0:1, :], 1.0)
            nc.vector.memset(negones[:], -1.0)
            nc.sync.dma_start(out=qc123[1:2, :], in_=negones[:])
            nc.sync.dma_start(out=qc0[1:2, :], in_=negones[0:1, 0:32])

            # ---------------- DMA issue: refs first, then features
            for l in (3, 2, 1, 0):
                for _, i, c0 in rt_chunks[l]:
                    Cc = min(128, LAYERS[l][0] - c0)
                    nc.sync.dma_start(out=rt[(l, i)][:],
                                      in_=rts[l][c0:c0 + Cc, :])

            # features, transposed fp8 [B, HW, C]; partition = hw-chunk
            C3, HW3 = LAYERS[3]
            f3 = pp.tile([49, B, C3], FP8, tag="f3", name="f3")
            nc.sync.dma_start(
                out=f3[:], in_=bass.AP(feats[3], 0,
                                       [[C3, 49], [HW3 * C3, B], [1, C3]]))

            C2, HW2 = LAYERS[2]
            f2 = pp.tile([98, B, 2, C2], FP8, tag="f2", name="f2")
            nc.sync.dma_start(
                out=f2[:], in_=bass.AP(feats[2], 0,
                                       [[2 * C2, 98], [HW2 * C2, B],
                                        [1, 2 * C2]]))

            C1, HW1 = LAYERS[1]   # 784 = 128*4 + 68*4
            f1a = pp.tile([128, B, 4, C1], FP8, tag="f1a", name="f1a")
            nc.sync.dma_start(
                out=f1a[:], in_=bass.AP(feats[1], 0,
                                        [[4 * C1, 128], [HW1 * C1, B],
                                         [1, 4 * C1]]))
            f1b = pp.tile([68, B, 4, C1], FP8, tag="f1b", name="f1b")
            nc.sync.dma_start(
                out=f1b[:], in_=bass.AP(feats[1], 512 * C1,
                                        [[4 * C1, 68], [HW1 * C1, B],
                                         [1, 4 * C1]]))

            C0, HW0 = LAYERS[0]   # 3136 = 3*1024 + 8*8; free = (pair, hw8, 2*C)
            f0 = [pp.tile([128, 16, 8, 2 * C0], FP8, tag=f"f0_{t}",
                          name=f"f0_{t}") for t in range(3)]
            for t in range(3):
                nc.sync.dma_start(
                    out=f0[t][:],
                    in_=bass.AP(feats[0], t * 1024 * 2 * C0,
                                [[16 * C0, 128], [HW0 * 2 * C0, 16],
                                 [1, 16 * C0]]))
            f0t = pp.tile([8, 16, 8, 2 * C0], FP8, tag="f0t", name="f0t")
            nc.sync.dma_start(
                out=f0t[:], in_=bass.AP(feats[0], 3072 * 2 * C0,
                                        [[16 * C0, 8], [HW0 * 2 * C0, 16],
                                         [1, 16 * C0]]))

            # ---------------- rn2 = sum_c r^2  (squares + ones matmul)
            for l in (3, 2, 1, 0):
                chunks = rt_chunks[l]
                sqs = []
                for _, i, c0 in chunks:
                    Cc = min(128, LAYERS[l][0] - c0)
                    sq = sqp.tile([128, R], BF16, tag="sq", name="sq")
                    sqs.append((sq, Cc))
                    if l >= 2:
                        nc.scalar.square(sq[0:Cc, :], rt[(l, i)][:])
                    else:
                        nc.gpsimd.tensor_tensor(sq[0:Cc, :], rt[(l, i)][:],
                                                rt[(l, i)][:],
                                                op=mybir.AluOpType.mult)
                for ci, (c0c, n) in enumerate(COL):
                    rn2ps = pR.tile([1, 512], F32, tag="rn2ps", name="rn2ps")
                    for i, (sq, Cc) in enumerate(sqs):
                        nc.tensor.matmul(rn2ps[0:1, 0:n],
                                         onesb[0:Cc, 0:1],
                                         sq[0:Cc, c0c:c0c + n],
                                         start=(i == 0),
                                         stop=(i == len(sqs) - 1))
                    if (l + ci) % 2 == 0:
                        nc.scalar.copy(rn2sb[0:1, l * R + c0c:l * R + c0c + n],
                                       rn2ps[0:1, 0:n])
                    else:
                        nc.vector.tensor_copy(
                            rn2sb[0:1, l * R + c0c:l * R + c0c + n],
                            rn2ps[0:1, 0:n])
            nc.sync.dma_start(out=rc_all[1:2, :], in_=rn2sb[:])

            # ---------------- pooling: q = sum_hw feat  (PE, N=1 matmuls)
            # qTearly cols: L3 a..d -> 0:128, L2 a/b -> 128:192, L1 -> 192:224
            # (cols 224:240 are the late L0 pooling output, written after the
            #  early-layer eviction below)
            qTe = pQ.tile([128, 240], F32, tag="qTe", name="qTe")
            qn2all = pQ.tile([1, 128], F32, tag="qn2all", name="qn2all")
            for j in range(B):
                s = int(PERM[j])
                for a in range(4):     # L3: lhsT [49, 128]
                    nc.tensor.matmul(qTe[:, 32 * a + j:32 * a + j + 1],
                                     f3[:, s, 128 * a:128 * (a + 1)],
                                     ones8[0:49, 0:1], start=True, stop=True)
            for j in range(B):
                s = int(PERM[j])
                for a in range(2):     # L2: lhsT [98, 128], 2 hw-subchunks
                    for h in range(2):
                        nc.tensor.matmul(
                            qTe[:, 128 + 32 * a + j:128 + 32 * a + j + 1],
                            f2[:, s, h, 128 * a:128 * (a + 1)],
                            ones8[0:98, 0:1], start=(h == 0), stop=(h == 1))
            for j in range(B):
                s = int(PERM[j])       # L1: 4+4 hw-subchunks
                for h in range(4):
                    nc.tensor.matmul(qTe[:, 192 + j:192 + j + 1],
                                     f1a[:, s, h, :], ones8[:, 0:1],
                                     start=(h == 0), stop=False)
                for h in range(4):
                    nc.tensor.matmul(qTe[:, 192 + j:192 + j + 1],
                                     f1b[:, s, h, :], ones8[0:68, 0:1],
                                     start=False, stop=(h == 3))

            # evict scaled 2q/... = 2*mean into fp8 qTs, per-layer scale
            qTs = pp.tile([128, 224], FP8, tag="qTs", name="qTs")
            nc.scalar.activation(qTs[:, 0:128], qTe[:, 0:128],
                                 mybir.ActivationFunctionType.Copy,
                                 scale=2.0 / HW3)
            nc.scalar.activation(qTs[:, 128:192], qTe[:, 128:192],
                                 mybir.ActivationFunctionType.Copy,
                                 scale=2.0 / HW2)
            nc.scalar.activation(qTs[:, 192:224], qTe[:, 192:224],
                                 mybir.ActivationFunctionType.Copy,
                                 scale=2.0 / HW1)

            # qn2neg rows for layers 1..3
            qsq = pp.tile([128, 224], BF16, tag="qsq", name="qsq")
            nc.scalar.activation(qsq[:], qTs[:],
                                 mybir.ActivationFunctionType.Square,
                                 scale=0.5)
            for l, dcol, cols in ((3, 0, [0, 32, 64, 96]),
                                  (2, 32, [128, 160]), (1, 64, [192])):
                for ai, a in enumerate(cols):
                    nc.tensor.matmul(qn2all[0:1, dcol:dcol + 32],
                                     onesb[:, 0:1], qsq[:, a:a + 32],
                                     start=(ai == 0), stop=(ai == len(cols) - 1))
                nc.scalar.activation(qc123[0:1, dcol:dcol + 32],
                                     qn2all[0:1, dcol:dcol + 32],
                                     mybir.ActivationFunctionType.Copy,
                                     scale=-1.0)

            # ---------------- distances for layers 3,2,1 (all col chunks)
            tbA = pp.tile([128, 1024], F32, tag="tbA", name="tbA")
            tbB = pp.tile([128, 976], F32, tag="tbB", name="tbB")

            def dist(l, ci):
                c0, n = COL[ci]
                dps = pD.tile([B, 512], F32, tag="dps", name="dps")
                if l == 0:
                    nc.tensor.matmul(dps[:, 0:n], qTs0c[:],
                                     rt[(0, 0)][:, c0:c0 + n],
                                     start=True, stop=False)
                elif l == 1:
                    nc.tensor.matmul(dps[:, 0:n], qTs[:, 192:224],
                                     rt[(1, 0)][:, c0:c0 + n],
                                     start=True, stop=False)
                else:
                    base = 0 if l == 3 else 128
                    for _, i, _c0 in rt_chunks[l]:
                        nc.tensor.matmul(dps[:, 0:n],
                                         qTs[:, base + 32 * i:base + 32 * i + 32],
                                         rt[(l, i)][:, c0:c0 + n],
                                         start=(i == 0), stop=False)
                # combo: row0 = -qn2 x ones, row1 = -1 x rn2
                lhs = (qc0[:] if l == 0 else
                       qc123[:, 32 * (3 - l):32 * (3 - l) + 32])
                nc.tensor.matmul(dps[:, 0:n], lhs,
                                 rc_all[:, l * R + c0:l * R + c0 + n],
                                 start=False, stop=True)
                dst = (tbA[32 * l:32 * l + 32, c0:c0 + n] if ci < 2 else
                       tbB[32 * l:32 * l + 32, c0 - 1024:c0 - 1024 + n])
                if (l + ci) % 2 == 0:
                    nc.scalar.copy(dst, dps[:, 0:n])
                else:
                    nc.vector.tensor_copy(dst, dps[:, 0:n])

            for ci in range(4):
                for l in (3, 2, 1):
                    dist(l, ci)

            # ---------------- pooling L0 (2-sample-packed lhsT [128, 128])
            for t in range(3):
                for p in range(16):
                    for h in range(8):
                        nc.tensor.matmul(qTe[:, 224 + p:225 + p],
                                         f0[t][:, p, h, :],
                                         ones8[:, 0:1],
                                         start=(t == 0 and h == 0), stop=False)
            for p in range(16):
                for h in range(8):
                    nc.tensor.matmul(qTe[:, 224 + p:225 + p],
                                     f0t[0:8, p, h, :],
                                     ones8[0:8, 0:1],
                                     start=False, stop=(h == 7))
            qTs0c = pp.tile([64, 32], FP8, tag="qTs0c", name="qTs0c")
            nc.scalar.activation(qTs0c[:, 0:16], qTe[0:64, 224:240],
                                 mybir.ActivationFunctionType.Copy,
                                 scale=2.0 / HW0)
            nc.scalar.activation(qTs0c[:, 16:32], qTe[64:128, 224:240],
                                 mybir.ActivationFunctionType.Copy,
                                 scale=2.0 / HW0)
            qsq0 = pp.tile([64, 32], BF16, tag="qsq0", name="qsq0")
            nc.scalar.activation(qsq0[:], qTs0c[:],
                                 mybir.ActivationFunctionType.Square,
                                 scale=0.5)
            nc.tensor.matmul(qn2all[0:1, 96:128], onesb[0:64, 0:1],
                             qsq0[:], start=True, stop=True)
            nc.scalar.activation(qc0[0:1, :], qn2all[0:1, 96:128],
                                 mybir.ActivationFunctionType.Copy,
                                 scale=-1.0)

            # ---------------- distances L0 + interleaved topk halves
            vals = pp.tile([128, 48], F32, tag="vals", name="vals")
            v24 = pp.tile([128, 24], F32, tag="v24", name="v24")

            def topk_half(tb, dstcol):
                nc.vector.max(vals[:, dstcol:dstcol + 8], tb[:])
                nc.vector.match_replace(tb[:], vals[:, dstcol:dstcol + 8],
                                        tb[:], NEG_BIG)
                nc.vector.max(vals[:, dstcol + 8:dstcol + 16], tb[:])
                nc.vector.match_replace(tb[:], vals[:, dstcol + 8:dstcol + 16],
                                        tb[:], NEG_BIG)
                nc.vector.max(vals[:, dstcol + 16:dstcol + 24], tb[:])

            dist(0, 0)
            dist(0, 1)
            topk_half(tbA, 0)
            dist(0, 2)
            dist(0, 3)
            topk_half(tbB, 24)

            # merge 48 -> 24
            nc.vector.max(v24[:, 0:8], vals[:])
            nc.vector.match_replace(vals[:], v24[:, 0:8], vals[:], NEG_BIG)
            nc.vector.max(v24[:, 8:16], vals[:])
            nc.vector.match_replace(vals[:], v24[:, 8:16], vals[:], NEG_BIG)
            nc.vector.max(v24[:, 16:24], vals[:])

            # ---------------- LID
            ln2 = pp.tile([128, 24], F32, tag="ln2", name="ln2")
            S = pp.tile([128, 1], F32, tag="S", name="S")
            denom = pp.tile([128, 1], F32, tag="denom", name="denom")
            lid = pp.tile([128, 1], F32, tag="lid", name="lid")
            nc.vector.tensor_scalar_min(v24[:], v24[:], -1e-30)
            nc.scalar.activation(ln2[:], v24[:],
                                 mybir.ActivationFunctionType.Ln, scale=-1.0)
            nc.vector.tensor_reduce(S[:], ln2[:, 1:21],
                                    axis=mybir.AxisListType.X,
                                    op=mybir.AluOpType.add)
            nc.vector.tensor_scalar(denom[:], ln2[:, 20:21], -20.0, S[:],
                                    op0=mybir.AluOpType.mult,
                                    op1=mybir.AluOpType.add)
            nc.vector.reciprocal(lid[:], denom[:])
            nc.vector.tensor_scalar_mul(lid[:], lid[:], -2.0 * K)

            # ---------------- regression + sigmoid
            lid4 = pp.tile([B, 4], F32, tag="lid4", name="lid4")
            for l in range(4):
                nc.vector.tensor_copy(lid4[:, l:l + 1],
                                      lid[32 * l:32 * l + 32, :])
            wps = pQ.tile([B, 5], F32, tag="wps", name="wps")
            nc.tensor.matmul(wps[:], ones_row[:], wb_sb[:],
                             start=True, stop=True)
            wbc = pp.tile([B, 5], F32, tag="wbc", name="wbc")
            nc.scalar.copy(wbc[:], wps[:])
            prod = pp.tile([B, 4], F32, tag="prod", name="prod")
            nc.vector.tensor_tensor(prod[:], lid4[:], wbc[:, 0:4],
                                    op=mybir.AluOpType.mult)
            ssum = pp.tile([B, 1], F32, tag="ssum", name="ssum")
            nc.vector.tensor_reduce(ssum[:], prod[:],
                                    axis=mybir.AxisListType.X,
                                    op=mybir.AluOpType.add)
            res = pp.tile([B, 1], F32, tag="res", name="res")
            nc.scalar.activation(res[:], ssum[:],
                                 mybir.ActivationFunctionType.Sigmoid,
                                 bias=wbc[:, 4:5])
            nc.sync.dma_start(out=out[:], in_=res[:])

    nc.compile()
    return nc


_NC = None


def _get_nc():
    global _NC
    if _NC is None:
        _NC = build_nc()
    return _NC


def run(trace=False, **inputs):
    nc = _get_nc()
    assert int(inputs.get("k", K)) == K

    # host prep: transpose to [B_full, HW, C] fp8 (layer 0 pair-interleaved
    # to [B_full/2, HW, 2, C])
    featsT = []
    for l, (C, HW) in enumerate(LAYERS):
        f = np.asarray(inputs[f"feat{l}"], dtype=np.float32)
        if l == 0:
            f = f.reshape(f.shape[0] // 2, 2, C, HW).transpose(0, 3, 1, 2)
            f = f.reshape(f.shape[0], HW, 2 * C)
        else:
            f = f.reshape(f.shape[0], C, HW).transpose(0, 2, 1)
        featsT.append(np.ascontiguousarray(f).astype(NP_FP8))
    rtsT = [np.ascontiguousarray(
        np.asarray(inputs[f"ref{l}"], dtype=np.float32).T).astype(NP_FP8)
        for l in range(4)]
    regw = np.asarray(inputs["reg_w"], dtype=np.float32).reshape(1, 4)
    regb = np.asarray(inputs["reg_b"], dtype=np.float32).reshape(1, 1)

    in_maps = []
    for c in range(N_CORES):
        m = {f"feat{l}": featsT[l][c * B:(c + 1) * B] for l in range(1, 4)}
        m["feat0"] = featsT[0][c * (B // 2):(c + 1) * (B // 2)]
        for l in range(4):
            m[f"rt{l}"] = rtsT[l]
        m["regw"] = regw
        m["regb"] = regb
        in_maps.append(m)

    res = run_bass_kernel_spmd(nc, in_maps, core_ids=list(range(N_CORES)),
                               trace=trace)
    full = np.empty((N_CORES * B,), dtype=np.float32)
    for c in range(N_CORES):
        shard = np.empty((B,), dtype=np.float32)
        shard[PERM] = res.results[c]["out"][:, 0]
        full[c * B:(c + 1) * B] = shard
    return full, res


def kernel(**inputs):
    return run(trace=False, **inputs)[0]


# revision 24
# speedup vs baseline: 2.2353x; 1.0470x over previous
"""LID detector kernel for Trainium2 (8 NeuronCores, data-parallel over batch).

Per core (batch shard of 32 samples):
  - features arrive host-transposed [B, HW, C] in fp8-e4m3; spatial mean
    pooling runs on the TensorEngine as ones-vector matmuls (reduction over
    the partition axis = hw), accumulating q directly in [C, B] layout in
    PSUM.  Layer 0 is pair-interleaved ([16, HW, 2*C]) so each matmul pools
    two samples.
  - reference tables arrive host-transposed [C, R] fp8; rn2 = ||r||^2 via
    square (ACT/gpsimd) + ones matmul.
  - -d2 = 2q.r - rn2 - qn2 accumulated fully in PSUM: C-chunk matmuls plus
    one K=2 matmul with lhsT [[-qn2],[-1]] and rhs [[ones],[rn2]]; eviction
    is then a plain copy into the topk buffer.
  - per-layer pipelines: each layer's pooling, qn2, and distance matmuls run
    as soon as its feature DMA lands; layer 1 is loaded last (cheapest tail).
  - top-24 smallest d2 via DVE max8 + match_replace on two column halves,
    then a 48-wide merge; LID = -2k / (sum_{i=2..21} ln d2_i - 20 ln d2_21).
  - logit = w . lid + b; sigmoid computed as 1/(1+exp(-logit)) so Ln and Exp
    share one ACT table set (preloaded by a dummy Ln at kernel start).
Sample order inside a core is PERM (evens then odds, from the layer-0 pair
packing); the host inverts it on gather.
"""

import sys

for _p in ("/opt/trn_rl_repo", "/root/.axon_site/_ro/trn_rl_repo"):
    if _p not in sys.path:
        sys.path.append(_p)

import ml_dtypes
import numpy as np

import concourse.mybir as mybir
from concourse import bass, bacc
from concourse.tile import TileContext
from concourse.bass_utils import run_bass_kernel_spmd

F32 = mybir.dt.float32
BF16 = mybir.dt.bfloat16
FP8 = mybir.dt.float8e4
NP_FP8 = ml_dtypes.float8_e4m3

N_CORES = 8
B = 32          # batch shard per core
R = 2000
K = 20
NEG_BIG = -3.0e38
LAYERS = [(64, 3136), (128, 784), (256, 196), (512, 49)]  # (C, H*W)

# column j of the on-device layout holds sample PERM[j] of the local shard
PERM = np.array([2 * j for j in range(16)] + [2 * j + 1 for j in range(16)])

# qTe column base per layer (layer 0 packed 2-per-column at 224:240)
QCOL = {3: 0, 2: 128, 1: 192, 0: 224}
# qc lhsT column base per layer within qcx [2, 128]
QCC = {3: 0, 2: 32, 1: 64, 0: 96}
COL = [(0, 512), (512, 512), (1024, 512), (1536, 464)]


def build_nc():
    nc = bacc.Bacc("TRN2", target_bir_lowering=False, debug=False,
                   num_devices=N_CORES)

    feats = [nc.dram_tensor(
        "feat0" if l == 0 else f"feat{l}",
        [B // 2, HW, 2 * C] if l == 0 else [B, HW, C],
        FP8, kind="ExternalInput") for l, (C, HW) in enumerate(LAYERS)]
    rts = [nc.dram_tensor(f"rt{l}", [C, R], FP8, kind="ExternalInput")
           for l, (C, _) in enumerate(LAYERS)]
    regw = nc.dram_tensor("regw", [1, 4], F32, kind="ExternalInput")
    regb = nc.dram_tensor("regb", [1, 1], F32, kind="ExternalInput")
    out = nc.dram_tensor("out", [B, 1], F32, kind="ExternalOutput")

    rt_chunks = {l: list(range(0, C, 128)) for l, (C, _) in enumerate(LAYERS)}

    with TileContext(nc) as tc:
        with (
            tc.tile_pool(name="pp", bufs=1) as pp,
            tc.tile_pool(name="sq", bufs=4) as sqp,
            tc.tile_pool(name="pR", bufs=2, space=bass.MemorySpace.PSUM) as pR,
            tc.tile_pool(name="pQ", bufs=1, space=bass.MemorySpace.PSUM) as pQ,
            tc.tile_pool(name="pD", bufs=3, space=bass.MemorySpace.PSUM) as pD,
        ):
            # ---------------- persistent SBUF tiles + constants
            ones8 = pp.tile([128, 1], FP8, tag="ones8", name="ones8")
            onesb = pp.tile([128, 1], BF16, tag="onesb", name="onesb")
            ones_row = pp.tile([1, B], F32, tag="ones_row", name="ones_row")
            dummy = pp.tile([1, 1], F32, tag="dummy", name="dummy")
            nc.vector.memset(ones8[:], 1.0)
            nc.vector.memset(onesb[:], 1.0)
            nc.vector.memset(ones_row[:], 1.0)
            nc.vector.memset(dummy[:], 1.0)
            # preload the natural_log/exp ACT table set; all later ACT ops
            # (copy/square/ln/exp) must stay within this set
            nc.scalar.activation(dummy[:], dummy[:],
                                 mybir.ActivationFunctionType.Ln)

            wb_sb = pp.tile([1, 5], F32, tag="wb_sb", name="wb_sb")
            nc.sync.dma_start(out=wb_sb[0:1, 0:4], in_=regw[:])
            nc.sync.dma_start(out=wb_sb[0:1, 4:5], in_=regb[:])

            rt = {}
            for l, (C, _) in enumerate(LAYERS):
                for i, c0 in enumerate(rt_chunks[l]):
                    Cc = min(128, C - c0)
                    rt[(l, i)] = pp.tile([Cc, R], FP8, tag=f"rt{l}_{i}",
                                         name=f"rt{l}_{i}")

            # combo-matmul operands (engines may only write partition 0; the
            # partition-1 rows are filled by small SBUF->SBUF DMAs):
            #   lhsT qcx: row0 = -qn2 per sample, row1 = -1
            #   rhs  rc_all: row0 = ones, row1 = rn2  (cols l*R + c)
            rn2sb = pp.tile([1, 4 * R], BF16, tag="rn2sb", name="rn2sb")
            rc_all = pp.tile([2, 4 * R], BF16, tag="rc_all", name="rc_all")
            qcx = pp.tile([2, 128], BF16, tag="qcx", name="qcx")
            negones = pp.tile([1, 128], BF16, tag="negones", name="negones")
            nc.vector.memset(rc_all[0:1, :], 1.0)
            nc.vector.memset(negones[:], -1.0)
            nc.sync.dma_start(out=qcx[1:2, :], in_=negones[:])

            # ---------------- DMA issue order = rough schedule
            for l in (0, 3, 2, 1):
                for i, c0 in enumerate(rt_chunks[l]):
                    Cc = min(128, LAYERS[l][0] - c0)
                    nc.sync.dma_start(out=rt[(l, i)][:],
                                      in_=rts[l][c0:c0 + Cc, :])

            C0, HW0 = LAYERS[0]   # 3136 = 3*1024 + 8*8; free = (pair, 8, 2C)
            f0 = [pp.tile([128, 16, 8, 2 * C0], FP8, tag=f"f0_{t}",
                          name=f"f0_{t}") for t in range(3)]
            for t in range(3):
                nc.sync.dma_start(
                    out=f0[t][:],
                    in_=bass.AP(feats[0], t * 1024 * 2 * C0,
                                [[16 * C0, 128], [HW0 * 2 * C0, 16],
                                 [1, 16 * C0]]))
            f0t = pp.tile([8, 16, 8, 2 * C0], FP8, tag="f0t", name="f0t")
            nc.sync.dma_start(
                out=f0t[:], in_=bass.AP(feats[0], 3072 * 2 * C0,
                                        [[16 * C0, 8], [HW0 * 2 * C0, 16],
                                         [1, 16 * C0]]))

            C3, HW3 = LAYERS[3]
            f3 = pp.tile([49, B, C3], FP8, tag="f3", name="f3")
            nc.sync.dma_start(
                out=f3[:], in_=bass.AP(feats[3], 0,
                                       [[C3, 49], [HW3 * C3, B], [1, C3]]))

            C2, HW2 = LAYERS[2]
            f2 = pp.tile([98, B, 2, C2], FP8, tag="f2", name="f2")
            nc.sync.dma_start(
                out=f2[:], in_=bass.AP(feats[2], 0,
                                       [[2 * C2, 98], [HW2 * C2, B],
                                        [1, 2 * C2]]))

            C1, HW1 = LAYERS[1]   # 784 = 128*4 + 68*4
            f1a = pp.tile([128, B, 4, C1], FP8, tag="f1a", name="f1a")
            nc.sync.dma_start(
                out=f1a[:], in_=bass.AP(feats[1], 0,
                                        [[4 * C1, 128], [HW1 * C1, B],
                                         [1, 4 * C1]]))
            f1b = pp.tile([68, B, 4, C1], FP8, tag="f1b", name="f1b")
            nc.sync.dma_start(
                out=f1b[:], in_=bass.AP(feats[1], 512 * C1,
                                        [[4 * C1, 68], [HW1 * C1, B],
                                         [1, 4 * C1]]))

            # ---------------- rn2 = sum_c r^2 (order: l0, l3 on ACT;
            # l2, l1 on gpsimd)
            for l, sq_eng in ((0, "act"), (3, "act"), (2, "gps"), (1, "gps")):
                chunks = rt_chunks[l]
                sqs = []
                for i, c0 in enumerate(chunks):
                    Cc = min(128, LAYERS[l][0] - c0)
                    sq = sqp.tile([128, R], BF16, tag="sq", name="sq")
                    sqs.append((sq, Cc))
                    if sq_eng == "act":
                        nc.scalar.square(sq[0:Cc, :], rt[(l, i)][:])
                    else:
                        nc.gpsimd.tensor_tensor(sq[0:Cc, :], rt[(l, i)][:],
                                                rt[(l, i)][:],
                                                op=mybir.AluOpType.mult)
                for ci, (c0c, n) in enumerate(COL):
                    rn2ps = pR.tile([1, 512], F32, tag="rn2ps", name="rn2ps")
                    for i, (sq, Cc) in enumerate(sqs):
                        nc.tensor.matmul(rn2ps[0:1, 0:n],
                                         onesb[0:Cc, 0:1],
                                         sq[0:Cc, c0c:c0c + n],
                                         start=(i == 0),
                                         stop=(i == len(sqs) - 1))
                    if ci % 2 == 0:
                        nc.scalar.copy(rn2sb[0:1, l * R + c0c:l * R + c0c + n],
                                       rn2ps[0:1, 0:n])
                    else:
                        nc.vector.tensor_copy(
                            rn2sb[0:1, l * R + c0c:l * R + c0c + n],
                            rn2ps[0:1, 0:n])
            nc.sync.dma_start(out=rc_all[1:2, :], in_=rn2sb[:])

            # ---------------- per-layer compute pipelines
            qTe = pQ.tile([128, 240], F32, tag="qTe", name="qTe")
            qn2all = pQ.tile([1, 128], F32, tag="qn2all", name="qn2all")
            qTs = pp.tile([128, 224], FP8, tag="qTs", name="qTs")
            qTs0c = pp.tile([64, 32], FP8, tag="qTs0c", name="qTs0c")
            qsq = pp.tile([128, 128], BF16, tag="qsq", name="qsq")
            tbA = pp.tile([128, 1024], F32, tag="tbA", name="tbA")
            tbB = pp.tile([128, 976], F32, tag="tbB", name="tbB")

            def pool_l0():
                for t in range(3):
                    for p in range(16):
                        for h in range(8):
                            nc.tensor.matmul(qTe[:, 224 + p:225 + p],
                                             f0[t][:, p, h, :],
                                             ones8[:, 0:1],
                                             start=(t == 0 and h == 0),
                                             stop=False)
                for p in range(16):
                    for h in range(8):
                        nc.tensor.matmul(qTe[:, 224 + p:225 + p],
                                         f0t[0:8, p, h, :],
                                         ones8[0:8, 0:1],
                                         start=False, stop=(h == 7))

            def pool_l3():
                for j in range(B):
                    s = int(PERM[j])
                    for a in range(4):
                        nc.tensor.matmul(qTe[:, 32 * a + j:32 * a + j + 1],
                                         f3[:, s, 128 * a:128 * (a + 1)],
                                         ones8[0:49, 0:1],
                                         start=True, stop=True)

            def pool_l2():
                for j in range(B):
                    s = int(PERM[j])
                    for a in range(2):
                        for h in range(2):
                            nc.tensor.matmul(
                                qTe[:, 128 + 32 * a + j:128 + 32 * a + j + 1],
                                f2[:, s, h, 128 * a:128 * (a + 1)],
                                ones8[0:98, 0:1],
                                start=(h == 0), stop=(h == 1))

            def pool_l1():
                for j in range(B):
                    s = int(PERM[j])
                    for h in range(4):
                        nc.tensor.matmul(qTe[:, 192 + j:192 + j + 1],
                                         f1a[:, s, h, :], ones8[:, 0:1],
                                         start=(h == 0), stop=False)
                    for h in range(4):
                        nc.tensor.matmul(qTe[:, 192 + j:192 + j + 1],
                                         f1b[:, s, h, :], ones8[0:68, 0:1],
                                         start=False, stop=(h == 3))

            def qphase(l):
                """evict scaled q, compute -qn2 into qcx; layer ready for
                distance matmuls afterwards"""
                C, HW = LAYERS[l]
                qb = QCOL[l]
                if l == 0:
                    nc.scalar.activation(qTs0c[:, 0:16], qTe[0:64, 224:240],
                                         mybir.ActivationFunctionType.Copy,
                                         scale=2.0 / HW)
                    nc.scalar.activation(qTs0c[:, 16:32], qTe[64:128, 224:240],
                                         mybir.ActivationFunctionType.Copy,
                                         scale=2.0 / HW)
                    nc.scalar.activation(qsq[0:64, 96:128], qTs0c[:],
                                         mybir.ActivationFunctionType.Square,
                                         scale=0.5)
                    nc.tensor.matmul(qn2all[0:1, 96:128], onesb[0:64, 0:1],
                                     qsq[0:64, 96:128], start=True, stop=True)
                    nc.scalar.activation(qcx[0:1, 96:128],
                                         qn2all[0:1, 96:128],
                                         mybir.ActivationFunctionType.Copy,
                                         scale=-1.0)
                    return
                nch = C // 128
                nc.scalar.activation(qTs[:, qb:qb + 32 * nch],
                                     qTe[:, qb:qb + 32 * nch],
                                     mybir.ActivationFunctionType.Copy,
                                     scale=2.0 / HW)
                nc.scalar.activation(qsq[:, 0:32 * nch], qTe[:, qb:qb + 32 * nch],
                                     mybir.ActivationFunctionType.Square,
                                     scale=1.0 / HW)
                for a in range(nch):
                    nc.tensor.matmul(qn2all[0:1, QCC[l]:QCC[l] + 32],
                                     onesb[:, 0:1],
                                     qsq[:, 32 * a:32 * a + 32],
                                     start=(a == 0), stop=(a == nch - 1))
                nc.scalar.activation(qcx[0:1, QCC[l]:QCC[l] + 32],
                                     qn2all[0:1, QCC[l]:QCC[l] + 32],
                                     mybir.ActivationFunctionType.Copy,
                                     scale=-1.0)

            def dist(l, ci, evict):
                c0, n = COL[ci]
                dps = pD.tile([B, 512], F32, tag="dps", name="dps")
                if l == 0:
                    nc.tensor.matmul(dps[:, 0:n], qTs0c[:],
                                     rt[(0, 0)][:, c0:c0 + n],
                                     start=True, stop=False)
                else:
                    qb = QCOL[l]
                    for i, _c0 in enumerate(rt_chunks[l]):
                        nc.tensor.matmul(dps[:, 0:n],
                                         qTs[:, qb + 32 * i:qb + 32 * i + 32],
                                         rt[(l, i)][:, c0:c0 + n],
                                         start=(i == 0), stop=False)
                nc.tensor.matmul(dps[:, 0:n],
                                 qcx[:, QCC[l]:QCC[l] + 32],
                                 rc_all[:, l * R + c0:l * R + c0 + n],
                                 start=False, stop=True)
                dst = (tbA[32 * l:32 * l + 32, c0:c0 + n] if ci < 2 else
                       tbB[32 * l:32 * l + 32, c0 - 1024:c0 - 1024 + n])
                if evict == "act":
                    nc.scalar.copy(dst, dps[:, 0:n])
                else:
                    nc.vector.tensor_copy(dst, dps[:, 0:n])

            # layer pipelines in data-arrival order; L1 last
            pool_l0()
            qphase(0)
            for ci in range(4):
                dist(0, ci, "act" if ci % 2 else "dve")
            pool_l3()
            qphase(3)
            for ci in range(4):
                dist(3, ci, "dve" if ci % 2 else "act")
            pool_l2()
            qphase(2)
            for ci in range(4):
                dist(2, ci, "act" if ci % 2 else "dve")
            pool_l1()
            qphase(1)

            vals = pp.tile([128, 48], F32, tag="vals", name="vals")
            v24 = pp.tile([128, 24], F32, tag="v24", name="v24")

            def topk_half(tb, dstcol):
                nc.vector.max(vals[:, dstcol:dstcol + 8], tb[:])
                nc.vector.match_replace(tb[:], vals[:, dstcol:dstcol + 8],
                                        tb[:], NEG_BIG)
                nc.vector.max(vals[:, dstcol + 8:dstcol + 16], tb[:])
                nc.vector.match_replace(tb[:], vals[:, dstcol + 8:dstcol + 16],
                                        tb[:], NEG_BIG)
                nc.vector.max(vals[:, dstcol + 16:dstcol + 24], tb[:])

            dist(1, 0, "act")
            dist(1, 1, "act")
            topk_half(tbA, 0)
            dist(1, 2, "act")
            dist(1, 3, "act")
            topk_half(tbB, 24)

            # merge 48 -> 24
            nc.vector.max(v24[:, 0:8], vals[:])
            nc.vector.match_replace(vals[:], v24[:, 0:8], vals[:], NEG_BIG)
            nc.vector.max(v24[:, 8:16], vals[:])
            nc.vector.match_replace(vals[:], v24[:, 8:16], vals[:], NEG_BIG)
            nc.vector.max(v24[:, 16:24], vals[:])

            # ---------------- LID
            ln2 = pp.tile([128, 24], F32, tag="ln2", name="ln2")
            S = pp.tile([128, 1], F32, tag="S", name="S")
            denom = pp.tile([128, 1], F32, tag="denom", name="denom")
            lid = pp.tile([128, 1], F32, tag="lid", name="lid")
            nc.vector.tensor_scalar_min(v24[:], v24[:], -1e-30)
            nc.scalar.activation(ln2[:], v24[:],
                                 mybir.ActivationFunctionType.Ln, scale=-1.0)
            nc.vector.tensor_reduce(S[:], ln2[:, 1:21],
                                    axis=mybir.AxisListType.X,
                                    op=mybir.AluOpType.add)
            nc.vector.tensor_scalar(denom[:], ln2[:, 20:21], -20.0, S[:],
                                    op0=mybir.AluOpType.mult,
                                    op1=mybir.AluOpType.add)
            nc.vector.reciprocal(lid[:], denom[:])
            nc.vector.tensor_scalar_mul(lid[:], lid[:], -2.0 * K)

            # ---------------- regression + sigmoid(x) = 1/(1+exp(-x))
            lid4 = pp.tile([B, 4], F32, tag="lid4", name="lid4")
            for l in range(4):
                nc.vector.tensor_copy(lid4[:, l:l + 1],
                                      lid[32 * l:32 * l + 32, :])
            wps = pQ.tile([B, 5], F32, tag="wps", name="wps")
            nc.tensor.matmul(wps[:], ones_row[:], wb_sb[:],
                             start=True, stop=True)
            wbc = pp.tile([B, 5], F32, tag="wbc", name="wbc")
            nc.scalar.copy(wbc[:], wps[:])
            prod = pp.tile([B, 4], F32, tag="prod", name="prod")
            nc.vector.tensor_tensor(prod[:], lid4[:], wbc[:, 0:4],
                                    op=mybir.AluOpType.mult)
            ssum = pp.tile([B, 1], F32, tag="ssum", name="ssum")
            nc.vector.tensor_reduce(ssum[:], prod[:],
                                    axis=mybir.AxisListType.X,
                                    op=mybir.AluOpType.add)
            # logit = ssum + b;  res = 1/(1 + exp(-logit))
            enx = pp.tile([B, 1], F32, tag="enx", name="enx")
            nc.vector.tensor_tensor(enx[:], ssum[:], wbc[:, 4:5],
                                    op=mybir.AluOpType.add)
            nc.scalar.activation(enx[:], enx[:],
                                 mybir.ActivationFunctionType.Exp, scale=-1.0)
            res = pp.tile([B, 1], F32, tag="res", name="res")
            nc.vector.tensor_scalar(res[:], enx[:], 1.0, None,
                                    op0=mybir.AluOpType.add)
            nc.vector.reciprocal(res[:], res[:])
            nc.sync.dma_start(out=out[:], in_=res[:])

    nc.compile()
    return nc


_NC = None


def _get_nc():
    global _NC
    if _NC is None:
        _NC = build_nc()
    return _NC


def run(trace=False, **inputs):
    nc = _get_nc()
    assert int(inputs.get("k", K)) == K

    # host prep: transpose to [B_full, HW, C] fp8 (layer 0 pair-interleaved
    # to [B_full/2, HW, 2*C])
    featsT = []
    for l, (C, HW) in enumerate(LAYERS):
        f = np.asarray(inputs[f"feat{l}"], dtype=np.float32)
        if l == 0:
            f = f.reshape(f.shape[0] // 2, 2, C, HW).transpose(0, 3, 1, 2)
            f = f.reshape(f.shape[0], HW, 2 * C)
        else:
            f = f.reshape(f.shape[0], C, HW).transpose(0, 2, 1)
        featsT.append(np.ascontiguousarray(f).astype(NP_FP8))
    rtsT = [np.ascontiguousarray(
        np.asarray(inputs[f"ref{l}"], dtype=np.float32).T).astype(NP_FP8)
        for l in range(4)]
    regw = np.asarray(inputs["reg_w"], dtype=np.float32).reshape(1, 4)
    regb = np.asarray(inputs["reg_b"], dtype=np.float32).reshape(1, 1)

    in_maps = []
    for c in range(N_CORES):
        m = {f"feat{l}": featsT[l][c * B:(c + 1) * B] for l in range(1, 4)}
        m["feat0"] = featsT[0][c * (B // 2):(c + 1) * (B // 2)]
        for l in range(4):
            m[f"rt{l}"] = rtsT[l]
        m["regw"] = regw
        m["regb"] = regb
        in_maps.append(m)

    res = run_bass_kernel_spmd(nc, in_maps, core_ids=list(range(N_CORES)),
                               trace=trace)
    full = np.empty((N_CORES * B,), dtype=np.float32)
    for c in range(N_CORES):
        shard = np.empty((B,), dtype=np.float32)
        shard[PERM] = res.results[c]["out"][:, 0]
        full[c * B:(c + 1) * B] = shard
    return full, res


def kernel(**inputs):
    return run(trace=False, **inputs)[0]


# revision 30
# speedup vs baseline: 2.2822x; 1.0210x over previous
"""LID detector kernel for Trainium2 (8 NeuronCores, data-parallel over batch).

Per core (batch shard of 32 samples):
  - features arrive host-transposed [B, HW, C] in fp8-e4m3; spatial mean
    pooling runs on the TensorEngine as ones-vector matmuls (reduction over
    the partition axis = hw), accumulating q directly in [C, B] layout in
    PSUM.  Layer 0 is pair-interleaved ([16, HW, 2*C]) so each matmul pools
    two samples.
  - reference tables arrive host-transposed [C, R] fp8; rn2 = ||r||^2 via
    square (ACT/gpsimd) + ones matmul.
  - -d2 = 2q.r - rn2 - qn2 accumulated fully in PSUM: C-chunk matmuls plus
    one K=2 matmul with lhsT [[-qn2],[-1]] and rhs [[ones],[rn2]]; eviction
    is then a plain copy into the topk buffer.
  - per-layer pipelines: each layer's pooling, qn2, and distance matmuls run
    as soon as its feature DMA lands; layer 1 is loaded last (cheapest tail).
  - top-24 smallest d2 via DVE max8 + match_replace on two column halves,
    then a 48-wide merge; LID = -2k / (sum_{i=2..21} ln d2_i - 20 ln d2_21).
  - logit = w . lid + b; sigmoid computed as 1/(1+exp(-logit)) so Ln and Exp
    share one ACT table set (preloaded by a dummy Ln at kernel start).
Sample order inside a core is PERM (evens then odds, from the layer-0 pair
packing); the host inverts it on gather.
"""

import sys

for _p in ("/opt/trn_rl_repo", "/root/.axon_site/_ro/trn_rl_repo"):
    if _p not in sys.path:
        sys.path.append(_p)

import ml_dtypes
import numpy as np

import concourse.mybir as mybir
from concourse import bass, bacc
from concourse.tile import TileContext
from concourse.bass_utils import run_bass_kernel_spmd

F32 = mybir.dt.float32
BF16 = mybir.dt.bfloat16
FP8 = mybir.dt.float8e4
NP_FP8 = ml_dtypes.float8_e4m3

N_CORES = 8
B = 32          # batch shard per core
R = 2000
K = 20
NEG_BIG = -3.0e38
LAYERS = [(64, 3136), (128, 784), (256, 196), (512, 49)]  # (C, H*W)

# column j of the on-device layout holds sample PERM[j] of the local shard
PERM = np.array([2 * j for j in range(16)] + [2 * j + 1 for j in range(16)])

# qTe column base per layer (layer 0 packed 2-per-column at 224:240)
QCOL = {3: 0, 2: 128, 1: 192, 0: 224}
# qc lhsT column base per layer within qcx [2, 128]
QCC = {3: 0, 2: 32, 1: 64, 0: 96}
COL = [(0, 512), (512, 512), (1024, 512), (1536, 464)]


def build_nc():
    nc = bacc.Bacc("TRN2", target_bir_lowering=False, debug=False,
                   num_devices=N_CORES)

    feats = [nc.dram_tensor(
        "feat0" if l == 0 else f"feat{l}",
        [B // 2, HW, 2 * C] if l == 0 else [B, HW, C],
        FP8, kind="ExternalInput") for l, (C, HW) in enumerate(LAYERS)]
    rts = [nc.dram_tensor(f"rt{l}", [C, R], FP8, kind="ExternalInput")
           for l, (C, _) in enumerate(LAYERS)]
    # regwb = [-2K * w, b]  (folded on the host)
    regwb = nc.dram_tensor("regwb", [1, 5], F32, kind="ExternalInput")
    out = nc.dram_tensor("out", [B, 1], F32, kind="ExternalOutput")

    rt_chunks = {l: list(range(0, C, 128)) for l, (C, _) in enumerate(LAYERS)}

    with TileContext(nc) as tc:
        with (
            tc.tile_pool(name="pp", bufs=1) as pp,
            tc.tile_pool(name="sq", bufs=4) as sqp,
            tc.tile_pool(name="pR", bufs=2, space=bass.MemorySpace.PSUM) as pR,
            tc.tile_pool(name="pQ", bufs=1, space=bass.MemorySpace.PSUM) as pQ,
            tc.tile_pool(name="pD", bufs=3, space=bass.MemorySpace.PSUM) as pD,
        ):
            # ---------------- persistent SBUF tiles + constants
            ones8 = pp.tile([128, 1], FP8, tag="ones8", name="ones8")
            onesb = pp.tile([128, 1], BF16, tag="onesb", name="onesb")
            ones_row = pp.tile([1, B], F32, tag="ones_row", name="ones_row")
            nc.vector.memset(ones8[:], 1.0)
            nc.vector.memset(onesb[:], 1.0)
            nc.vector.memset(ones_row[:], 1.0)
            # preload the natural_log_exp_and_others ACT table set; all ACT
            # ops used below (copy/square/ln/exp) live in this one set, so no
            # further table loads are inserted
            nc.scalar.add_instruction(mybir.InstLoadActFuncSet(
                name=nc.get_next_instruction_name(), ins=[], outs=[],
                act_func_set_id=6))

            wb_sb = pp.tile([1, 5], F32, tag="wb_sb", name="wb_sb")
            nc.sync.dma_start(out=wb_sb[:], in_=regwb[:])

            rt = {}
            for l, (C, _) in enumerate(LAYERS):
                for i, c0 in enumerate(rt_chunks[l]):
                    Cc = min(128, C - c0)
                    rt[(l, i)] = pp.tile([Cc, R], FP8, tag=f"rt{l}_{i}",
                                         name=f"rt{l}_{i}")

            # combo-matmul operands (engines may only write partition 0; the
            # partition-1 rows are filled by small SBUF->SBUF DMAs):
            #   lhsT qcx: row0 = -qn2 per sample, row1 = -1
            #   rhs  rc_all: row0 = ones, row1 = rn2  (cols l*R + c)
            rn2sb = pp.tile([1, 4 * R], BF16, tag="rn2sb", name="rn2sb")
            rc_all = pp.tile([2, 4 * R], BF16, tag="rc_all", name="rc_all")
            qcx = pp.tile([2, 128], BF16, tag="qcx", name="qcx")
            negones = pp.tile([1, 128], BF16, tag="negones", name="negones")
            nc.vector.memset(rc_all[0:1, :], 1.0)
            nc.vector.memset(negones[:], -1.0)
            nc.sync.dma_start(out=qcx[1:2, :], in_=negones[:])

            # ---------------- DMA issue order = rough schedule
            for l in (0, 3, 2, 1):
                for i, c0 in enumerate(rt_chunks[l]):
                    Cc = min(128, LAYERS[l][0] - c0)
                    nc.sync.dma_start(out=rt[(l, i)][:],
                                      in_=rts[l][c0:c0 + Cc, :])

            C0, HW0 = LAYERS[0]   # 3136 = 3*1024 + 8*8; free = (pair, 8, 2C)
            f0 = [pp.tile([128, 16, 8, 2 * C0], FP8, tag=f"f0_{t}",
                          name=f"f0_{t}") for t in range(3)]
            for t in range(3):
                nc.sync.dma_start(
                    out=f0[t][:],
                    in_=bass.AP(feats[0], t * 1024 * 2 * C0,
                                [[16 * C0, 128], [HW0 * 2 * C0, 16],
                                 [1, 16 * C0]]))
            f0t = pp.tile([8, 16, 8, 2 * C0], FP8, tag="f0t", name="f0t")
            nc.sync.dma_start(
                out=f0t[:], in_=bass.AP(feats[0], 3072 * 2 * C0,
                                        [[16 * C0, 8], [HW0 * 2 * C0, 16],
                                         [1, 16 * C0]]))

            C3, HW3 = LAYERS[3]
            f3 = pp.tile([49, B, C3], FP8, tag="f3", name="f3")
            nc.sync.dma_start(
                out=f3[:], in_=bass.AP(feats[3], 0,
                                       [[C3, 49], [HW3 * C3, B], [1, C3]]))

            C2, HW2 = LAYERS[2]
            f2 = pp.tile([98, B, 2, C2], FP8, tag="f2", name="f2")
            nc.sync.dma_start(
                out=f2[:], in_=bass.AP(feats[2], 0,
                                       [[2 * C2, 98], [HW2 * C2, B],
                                        [1, 2 * C2]]))

            C1, HW1 = LAYERS[1]   # 784 = 128*4 + 68*4
            f1a = pp.tile([128, B, 4, C1], FP8, tag="f1a", name="f1a")
            nc.sync.dma_start(
                out=f1a[:], in_=bass.AP(feats[1], 0,
                                        [[4 * C1, 128], [HW1 * C1, B],
                                         [1, 4 * C1]]))
            f1b = pp.tile([68, B, 4, C1], FP8, tag="f1b", name="f1b")
            nc.sync.dma_start(
                out=f1b[:], in_=bass.AP(feats[1], 512 * C1,
                                        [[4 * C1, 68], [HW1 * C1, B],
                                         [1, 4 * C1]]))

            # ---------------- rn2 = sum_c r^2 (order: l0, l3 on ACT;
            # l2, l1 on gpsimd)
            for l, sq_eng in ((0, "act"), (3, "act"), (2, "gps"), (1, "gps")):
                chunks = rt_chunks[l]
                sqs = []
                for i, c0 in enumerate(chunks):
                    Cc = min(128, LAYERS[l][0] - c0)
                    sq = sqp.tile([128, R], BF16, tag="sq", name="sq")
                    sqs.append((sq, Cc))
                    if sq_eng == "act":
                        nc.scalar.square(sq[0:Cc, :], rt[(l, i)][:])
                    else:
                        nc.gpsimd.tensor_tensor(sq[0:Cc, :], rt[(l, i)][:],
                                                rt[(l, i)][:],
                                                op=mybir.AluOpType.mult)
                for ci, (c0c, n) in enumerate(COL):
                    rn2ps = pR.tile([1, 512], F32, tag="rn2ps", name="rn2ps")
                    for i, (sq, Cc) in enumerate(sqs):
                        nc.tensor.matmul(rn2ps[0:1, 0:n],
                                         onesb[0:Cc, 0:1],
                                         sq[0:Cc, c0c:c0c + n],
                                         start=(i == 0),
                                         stop=(i == len(sqs) - 1))
                    if ci % 2 == 0:
                        nc.scalar.copy(rn2sb[0:1, l * R + c0c:l * R + c0c + n],
                                       rn2ps[0:1, 0:n])
                    else:
                        nc.vector.tensor_copy(
                            rn2sb[0:1, l * R + c0c:l * R + c0c + n],
                            rn2ps[0:1, 0:n])
            nc.sync.dma_start(out=rc_all[1:2, :], in_=rn2sb[:])

            # ---------------- per-layer compute pipelines
            qTe = pQ.tile([128, 240], F32, tag="qTe", name="qTe")
            qn2all = pQ.tile([1, 128], F32, tag="qn2all", name="qn2all")
            qTs = pp.tile([128, 224], FP8, tag="qTs", name="qTs")
            qTs0c = pp.tile([64, 32], FP8, tag="qTs0c", name="qTs0c")
            qsq = pp.tile([128, 128], BF16, tag="qsq", name="qsq")
            tbA = pp.tile([128, 1024], F32, tag="tbA", name="tbA")
            tbB = pp.tile([128, 976], F32, tag="tbB", name="tbB")

            def pool_l0():
                for t in range(3):
                    for p in range(16):
                        for h in range(8):
                            nc.tensor.matmul(qTe[:, 224 + p:225 + p],
                                             f0[t][:, p, h, :],
                                             ones8[:, 0:1],
                                             start=(t == 0 and h == 0),
                                             stop=False)
                for p in range(16):
                    for h in range(8):
                        nc.tensor.matmul(qTe[:, 224 + p:225 + p],
                                         f0t[0:8, p, h, :],
                                         ones8[0:8, 0:1],
                                         start=False, stop=(h == 7))

            def pool_l3():
                for j in range(B):
                    s = int(PERM[j])
                    for a in range(4):
                        nc.tensor.matmul(qTe[:, 32 * a + j:32 * a + j + 1],
                                         f3[:, s, 128 * a:128 * (a + 1)],
                                         ones8[0:49, 0:1],
                                         start=True, stop=True)

            def pool_l2():
                for j in range(B):
                    s = int(PERM[j])
                    for a in range(2):
                        for h in range(2):
                            nc.tensor.matmul(
                                qTe[:, 128 + 32 * a + j:128 + 32 * a + j + 1],
                                f2[:, s, h, 128 * a:128 * (a + 1)],
                                ones8[0:98, 0:1],
                                start=(h == 0), stop=(h == 1))

            def pool_l1():
                # all f1a matmuls first: the f1b DMA lands last and must not
                # block the in-order PE queue
                for j in range(B):
                    s = int(PERM[j])
                    for h in range(4):
                        nc.tensor.matmul(qTe[:, 192 + j:192 + j + 1],
                                         f1a[:, s, h, :], ones8[:, 0:1],
                                         start=(h == 0), stop=False)
                for j in range(B):
                    s = int(PERM[j])
                    for h in range(4):
                        nc.tensor.matmul(qTe[:, 192 + j:192 + j + 1],
                                         f1b[:, s, h, :], ones8[0:68, 0:1],
                                         start=False, stop=(h == 3))

            def qphase(l):
                """evict scaled q, compute -qn2 into qcx; layer ready for
                distance matmuls afterwards"""
                C, HW = LAYERS[l]
                qb = QCOL[l]
                if l == 0:
                    nc.scalar.activation(qTs0c[:, 0:16], qTe[0:64, 224:240],
                                         mybir.ActivationFunctionType.Copy,
                                         scale=2.0 / HW)
                    nc.scalar.activation(qTs0c[:, 16:32], qTe[64:128, 224:240],
                                         mybir.ActivationFunctionType.Copy,
                                         scale=2.0 / HW)
                    nc.scalar.activation(qsq[0:64, 96:128], qTs0c[:],
                                         mybir.ActivationFunctionType.Square,
                                         scale=0.5)
                    nc.tensor.matmul(qn2all[0:1, 96:128], onesb[0:64, 0:1],
                                     qsq[0:64, 96:128], start=True, stop=True)
                    nc.scalar.activation(qcx[0:1, 96:128],
                                         qn2all[0:1, 96:128],
                                         mybir.ActivationFunctionType.Copy,
                                         scale=-1.0)
                    return
                nch = C // 128
                nc.scalar.activation(qTs[:, qb:qb + 32 * nch],
                                     qTe[:, qb:qb + 32 * nch],
                                     mybir.ActivationFunctionType.Copy,
                                     scale=2.0 / HW)
                nc.scalar.activation(qsq[:, 0:32 * nch], qTe[:, qb:qb + 32 * nch],
                                     mybir.ActivationFunctionType.Square,
                                     scale=1.0 / HW)
                for a in range(nch):
                    nc.tensor.matmul(qn2all[0:1, QCC[l]:QCC[l] + 32],
                                     onesb[:, 0:1],
                                     qsq[:, 32 * a:32 * a + 32],
                                     start=(a == 0), stop=(a == nch - 1))
                nc.scalar.activation(qcx[0:1, QCC[l]:QCC[l] + 32],
                                     qn2all[0:1, QCC[l]:QCC[l] + 32],
                                     mybir.ActivationFunctionType.Copy,
                                     scale=-1.0)

            def dist(l, ci, evict):
                c0, n = COL[ci]
                dps = pD.tile([B, 512], F32, tag="dps", name="dps")
                if l == 0:
                    nc.tensor.matmul(dps[:, 0:n], qTs0c[:],
                                     rt[(0, 0)][:, c0:c0 + n],
                                     start=True, stop=False)
                else:
                    qb = QCOL[l]
                    for i, _c0 in enumerate(rt_chunks[l]):
                        nc.tensor.matmul(dps[:, 0:n],
                                         qTs[:, qb + 32 * i:qb + 32 * i + 32],
                                         rt[(l, i)][:, c0:c0 + n],
                                         start=(i == 0), stop=False)
                nc.tensor.matmul(dps[:, 0:n],
                                 qcx[:, QCC[l]:QCC[l] + 32],
                                 rc_all[:, l * R + c0:l * R + c0 + n],
                                 start=False, stop=True)
                dst = (tbA[32 * l:32 * l + 32, c0:c0 + n] if ci < 2 else
                       tbB[32 * l:32 * l + 32, c0 - 1024:c0 - 1024 + n])
                if evict == "act":
                    nc.scalar.copy(dst, dps[:, 0:n])
                else:
                    nc.vector.tensor_copy(dst, dps[:, 0:n])

            # layer pipelines in data-arrival order; L1 last
            pool_l0()
            qphase(0)
            for ci in range(4):
                dist(0, ci, "act" if ci % 2 else "dve")
            pool_l3()
            qphase(3)
            for ci in range(4):
                dist(3, ci, "dve" if ci % 2 else "act")
            pool_l2()
            qphase(2)
            for ci in range(4):
                dist(2, ci, "act" if ci % 2 else "dve")
            pool_l1()
            qphase(1)

            vals = pp.tile([128, 48], F32, tag="vals", name="vals")
            v24 = pp.tile([128, 24], F32, tag="v24", name="v24")

            def topk_half(tb, dstcol):
                nc.vector.max(vals[:, dstcol:dstcol + 8], tb[:])
                nc.vector.match_replace(tb[:], vals[:, dstcol:dstcol + 8],
                                        tb[:], NEG_BIG)
                nc.vector.max(vals[:, dstcol + 8:dstcol + 16], tb[:])
                nc.vector.match_replace(tb[:], vals[:, dstcol + 8:dstcol + 16],
                                        tb[:], NEG_BIG)
                nc.vector.max(vals[:, dstcol + 16:dstcol + 24], tb[:])

            dist(1, 0, "act")
            dist(1, 1, "act")
            topk_half(tbA, 0)
            dist(1, 2, "act")
            dist(1, 3, "act")
            topk_half(tbB, 24)

            # merge 48 -> 24
            nc.vector.max(v24[:, 0:8], vals[:])
            nc.vector.match_replace(vals[:], v24[:, 0:8], vals[:], NEG_BIG)
            nc.vector.max(v24[:, 8:16], vals[:])
            nc.vector.match_replace(vals[:], v24[:, 8:16], vals[:], NEG_BIG)
            nc.vector.max(v24[:, 16:24], vals[:])

            # ---------------- LID
            ln2 = pp.tile([128, 24], F32, tag="ln2", name="ln2")
            S = pp.tile([128, 1], F32, tag="S", name="S")
            denom = pp.tile([128, 1], F32, tag="denom", name="denom")
            lid = pp.tile([128, 1], F32, tag="lid", name="lid")
            nc.vector.tensor_scalar_min(v24[:], v24[:], -1e-30)
            nc.scalar.activation(ln2[:], v24[:],
                                 mybir.ActivationFunctionType.Ln, scale=-1.0)
            nc.vector.tensor_reduce(S[:], ln2[:, 1:21],
                                    axis=mybir.AxisListType.X,
                                    op=mybir.AluOpType.add)
            nc.vector.tensor_scalar(denom[:], ln2[:, 20:21], -20.0, S[:],
                                    op0=mybir.AluOpType.mult,
                                    op1=mybir.AluOpType.add)
            # lid holds 1/denom; the -2K factor is folded into the host-side
            # regression weights
            nc.vector.reciprocal(lid[:], denom[:])

            # ---------------- regression + sigmoid(x) = 1/(1+exp(-x))
            lid4 = pp.tile([B, 4], F32, tag="lid4", name="lid4")
            for l in range(4):
                nc.vector.tensor_copy(lid4[:, l:l + 1],
                                      lid[32 * l:32 * l + 32, :])
            wps = pQ.tile([B, 5], F32, tag="wps", name="wps")
            nc.tensor.matmul(wps[:], ones_row[:], wb_sb[:],
                             start=True, stop=True)
            wbc = pp.tile([B, 5], F32, tag="wbc", name="wbc")
            nc.scalar.copy(wbc[:], wps[:])
            prod = pp.tile([B, 4], F32, tag="prod", name="prod")
            nc.vector.tensor_tensor(prod[:], lid4[:], wbc[:, 0:4],
                                    op=mybir.AluOpType.mult)
            ssum = pp.tile([B, 1], F32, tag="ssum", name="ssum")
            nc.vector.tensor_reduce(ssum[:], prod[:],
                                    axis=mybir.AxisListType.X,
                                    op=mybir.AluOpType.add)
            # logit = ssum + b;  res = 1/(1 + exp(-logit))
            enx = pp.tile([B, 1], F32, tag="enx", name="enx")
            nc.vector.tensor_tensor(enx[:], ssum[:], wbc[:, 4:5],
                                    op=mybir.AluOpType.add)
            nc.scalar.activation(enx[:], enx[:],
                                 mybir.ActivationFunctionType.Exp, scale=-1.0)
            res = pp.tile([B, 1], F32, tag="res", name="res")
            nc.vector.tensor_scalar(res[:], enx[:], 1.0, None,
                                    op0=mybir.AluOpType.add)
            nc.vector.reciprocal(res[:], res[:])
            nc.sync.dma_start(out=out[:], in_=res[:])

    nc.compile()
    return nc


_NC = None


def _get_nc():
    global _NC
    if _NC is None:
        _NC = build_nc()
    return _NC


def run(trace=False, **inputs):
    nc = _get_nc()
    assert int(inputs.get("k", K)) == K

    # host prep: transpose to [B_full, HW, C] fp8 (layer 0 pair-interleaved
    # to [B_full/2, HW, 2*C])
    featsT = []
    for l, (C, HW) in enumerate(LAYERS):
        f = np.asarray(inputs[f"feat{l}"], dtype=np.float32)
        if l == 0:
            f = f.reshape(f.shape[0] // 2, 2, C, HW).transpose(0, 3, 1, 2)
            f = f.reshape(f.shape[0], HW, 2 * C)
        else:
            f = f.reshape(f.shape[0], C, HW).transpose(0, 2, 1)
        featsT.append(np.ascontiguousarray(f).astype(NP_FP8))
    rtsT = [np.ascontiguousarray(
        np.asarray(inputs[f"ref{l}"], dtype=np.float32).T).astype(NP_FP8)
        for l in range(4)]
    regw = np.asarray(inputs["reg_w"], dtype=np.float32).reshape(1, 4)
    regb = np.asarray(inputs["reg_b"], dtype=np.float32).reshape(1, 1)
    regwb = np.concatenate([regw * (-2.0 * K), regb], axis=1)

    in_maps = []
    for c in range(N_CORES):
        m = {f"feat{l}": featsT[l][c * B:(c + 1) * B] for l in range(1, 4)}
        m["feat0"] = featsT[0][c * (B // 2):(c + 1) * (B // 2)]
        for l in range(4):
            m[f"rt{l}"] = rtsT[l]
        m["regwb"] = regwb
        in_maps.append(m)

    res = run_bass_kernel_spmd(nc, in_maps, core_ids=list(range(N_CORES)),
                               trace=trace)
    full = np.empty((N_CORES * B,), dtype=np.float32)
    for c in range(N_CORES):
        shard = np.empty((B,), dtype=np.float32)
        shard[PERM] = res.results[c]["out"][:, 0]
        full[c * B:(c + 1) * B] = shard
    return full, res


def kernel(**inputs):
    return run(trace=False, **inputs)[0]
